# revision 1
# baseline (speedup 1.0000x reference)
"""MultiDense / MoE-routing kernel for 8 Trainium2 NeuronCores.

out[b, s, :] = values[b, s, :] @ W[lookups[b]] + bias[lookups[b]]

Strategy: route on host, compute on device.
  * Experts are sorted by sample count and grouped into 8 "slots" of 8
    similar-sized experts; slot s's experts go one-per-core, so each core
    holds only 8 expert weight matrices (0.5 MiB) and ~1/8 of the batch.
  * Per-core activations are gathered into slot-contiguous order and
    pre-transposed to [IN_DIM=128 partitions, tokens] on host so the device
    does dense matmuls only (no gathers, no transposes).
  * Slot capacities (max expert count within a slot group) are baked into
    the compiled Bass program; all 8 cores run the same SPMD program.
Device per chunk of 512 token-columns: DMA in -> per-slot fp32 matmul
(out^T = W_d^T @ X^T accumulated in PSUM) -> bias-add fused with the
PSUM->SBUF copy -> DMA out. Host inverse-permutes the result.
"""

import numpy as np

import concourse.bacc as bacc
import concourse.mybir as mybir
from concourse import bass, tile
from concourse.bass_utils import run_bass_kernel_spmd

B, S, IN_DIM, OUT_DIM, DIMS = 16384, 4, 128, 128, 64
N_CORES = 8
CHUNK = 512
F32 = mybir.dt.float32


def _make_plan(lookups):
    """Decide expert->(core,slot) placement and slot capacities."""
    counts = np.bincount(lookups, minlength=DIMS)
    order = np.argsort(-counts, kind="stable")

    # Expert-sharded: slot s holds experts ranked [8s, 8s+8), one per core.
    caps_es = [int(counts[order[8 * s]]) for s in range(8)]
    t_es = S * sum(caps_es)
    bytes_es = 2 * t_es * IN_DIM * 4 + 8 * IN_DIM * OUT_DIM * 4

    # Data-parallel: every core holds all 64 experts, samples split 8 ways.
    caps_dp = [int(np.ceil(counts[d] / N_CORES)) for d in range(DIMS)]
    t_dp = S * sum(caps_dp)
    bytes_dp = 2 * t_dp * IN_DIM * 4 + DIMS * IN_DIM * OUT_DIM * 4

    by_expert = [np.nonzero(lookups == d)[0] for d in range(DIMS)]
    if bytes_es <= bytes_dp:
        caps = caps_es
        slot_experts = [[int(order[8 * s + c]) for s in range(8)]
                        for c in range(N_CORES)]
        slot_samples = [[by_expert[slot_experts[c][s]] for s in range(8)]
                        for c in range(N_CORES)]
    else:
        caps = caps_dp
        slot_experts = [list(range(DIMS)) for _ in range(N_CORES)]
        slot_samples = [[by_expert[d][c::N_CORES] for d in range(DIMS)]
                        for c in range(N_CORES)]
    return caps, slot_experts, slot_samples


def _build_program(caps):
    """One SPMD Bass program; slot boundaries from `caps` are hardcoded."""
    n_slots = len(caps)
    total = S * sum(caps)  # token columns per core, incl. padding
    segs = []  # (slot, col0, col1)
    off = 0
    for s, cap in enumerate(caps):
        segs.append((s, off, off + S * cap))
        off += S * cap

    nc = bacc.Bacc("TRN2", target_bir_lowering=False, debug=False,
                   num_devices=N_CORES)
    xt = nc.dram_tensor("xt", [IN_DIM, total], F32, kind="ExternalInput")
    wt = nc.dram_tensor("wt", [IN_DIM, n_slots * OUT_DIM], F32,
                        kind="ExternalInput")
    bt = nc.dram_tensor("bt", [OUT_DIM, n_slots], F32, kind="ExternalInput")
    ot = nc.dram_tensor("ot", [OUT_DIM, total], F32, kind="ExternalOutput")

    n_wgroups = (n_slots + 7) // 8
    with tile.TileContext(nc) as tc:
        with (
            tc.tile_pool(name="wpool", bufs=1) as wpool,
            tc.tile_pool(name="xpool", bufs=4) as xpool,
            tc.tile_pool(name="opool", bufs=4) as opool,
            tc.tile_pool(name="pspool", bufs=4, space="PSUM") as pspool,
        ):
            wtiles = []
            for g in range(n_wgroups):
                gw = min(8, n_slots - 8 * g) * OUT_DIM
                wtile = wpool.tile([IN_DIM, gw], F32, tag=f"w{g}")
                nc.sync.dma_start(wtile[:],
                                  wt[:, 8 * g * OUT_DIM:8 * g * OUT_DIM + gw])
                wtiles.append(wtile)
            btile = wpool.tile([OUT_DIM, n_slots], F32, tag="bias")
            nc.sync.dma_start(btile[:], bt[:])

            n_pieces = 0
            for c0 in range(0, total, CHUNK):
                c1 = min(c0 + CHUNK, total)
                w = c1 - c0
                xtile = xpool.tile([IN_DIM, w], F32, tag="x")
                nc.sync.dma_start(xtile[:], xt[:, c0:c1])
                ps = pspool.tile([OUT_DIM, w], F32, tag="ps")
                otile = opool.tile([OUT_DIM, w], F32, tag="o")
                pieces = []
                for s, s0, s1 in segs:
                    lc, rc = max(c0, s0), min(c1, s1)
                    if lc < rc:
                        pieces.append((s, lc - c0, rc - c0))
                for s, l, r in pieces:
                    nc.tensor.matmul(
                        ps[:, l:r],
                        wtiles[s // 8][:, (s % 8) * OUT_DIM:(s % 8 + 1) * OUT_DIM],
                        xtile[:, l:r],
                        start=True, stop=True,
                    )
                for s, l, r in pieces:
                    if n_pieces % 2 == 0:
                        nc.vector.tensor_scalar_add(
                            otile[:, l:r], ps[:, l:r], btile[:, s:s + 1])
                    else:
                        nc.scalar.activation(
                            otile[:, l:r], ps[:, l:r],
                            mybir.ActivationFunctionType.Identity,
                            bias=btile[:, s:s + 1], scale=1.0)
                    n_pieces += 1
                nc.sync.dma_start(ot[:, c0:c1], otile[:])
    nc.compile()
    return nc


def _run(values, W, b, lookups, trace=False, tmpdir=None):
    values = np.asarray(values, dtype=np.float32)
    W = np.asarray(W, dtype=np.float32)
    b = np.asarray(b, dtype=np.float32)
    lookups = np.asarray(lookups).astype(np.int64)

    caps, slot_experts, slot_samples = _make_plan(lookups)
    n_slots = len(caps)
    n_samp = sum(caps)  # padded sample rows per core
    total = S * n_samp

    slot_off = np.cumsum([0] + caps[:-1])
    in_maps = []
    placements = []  # per core: (src_idx, dst_rows)
    for c in range(N_CORES):
        Xg = np.zeros((n_samp, S, IN_DIM), np.float32)
        src_all, dst_all = [], []
        for s in range(n_slots):
            idxs = slot_samples[c][s]
            if len(idxs):
                rows = np.arange(slot_off[s], slot_off[s] + len(idxs))
                Xg[rows] = values[idxs]
                src_all.append(idxs)
                dst_all.append(rows)
        src_all = np.concatenate(src_all) if src_all else np.empty(0, np.int64)
        dst_all = np.concatenate(dst_all) if dst_all else np.empty(0, np.int64)
        placements.append((src_all, dst_all))

        experts_c = slot_experts[c]
        wt = np.ascontiguousarray(
            W[experts_c].transpose(1, 0, 2).reshape(IN_DIM, n_slots * OUT_DIM))
        bt = np.ascontiguousarray(b[experts_c].T)
        xt = np.ascontiguousarray(Xg.reshape(total, IN_DIM).T)
        in_maps.append({"xt": xt, "wt": wt, "bt": bt})

    nc = _build_program(caps)
    res = run_bass_kernel_spmd(nc, in_maps, list(range(N_CORES)),
                               trace=trace, tmpdir=tmpdir)

    out = np.empty((B, S, OUT_DIM), np.float32)
    for c in range(N_CORES):
        og = np.ascontiguousarray(res.results[c]["ot"].T).reshape(
            n_samp, S, OUT_DIM)
        src_all, dst_all = placements[c]
        out[src_all] = og[dst_all]
    return out, res


def kernel(values, W, b, lookups):
    out, _ = _run(values, W, b, lookups)
    return out


# revision 3
# speedup vs baseline: 1.2553x; 1.2553x over previous
"""MultiDense / MoE-routing kernel for 8 Trainium2 NeuronCores.

out[b, s, :] = values[b, s, :] @ W[lookups[b]] + bias[lookups[b]]

Strategy: route on host, compute on device.
  * Experts are sorted by sample count and grouped into 8 "slots" of 8
    similar-sized experts; slot s's experts go one-per-core, so each core
    holds only 8 expert weight matrices (0.5 MiB) and ~1/8 of the batch.
  * Per-core activations are gathered into slot-contiguous order and
    pre-transposed to [IN_DIM=128 partitions, tokens] on host so the device
    does dense matmuls only (no gathers, no transposes).
  * Slot capacities (max expert count within a slot group) are baked into
    the compiled Bass program; all 8 cores run the same SPMD program.
Device per chunk of 512 token-columns: DMA in -> per-slot fp32 matmul
(out^T = W_d^T @ X^T accumulated in PSUM) -> bias-add fused with the
PSUM->SBUF copy -> DMA out. Host inverse-permutes the result.
"""

import numpy as np

import concourse.bacc as bacc
import concourse.mybir as mybir
from concourse import bass, tile
from concourse.bass_utils import run_bass_kernel_spmd

B, S, IN_DIM, OUT_DIM, DIMS = 16384, 4, 128, 128, 64
N_CORES = 8
CHUNK = 2048          # token-columns per pipeline chunk (4 PSUM banks)
MM_N = 512            # max fp32 matmul moving free dim = 1 PSUM bank
F32 = mybir.dt.float32


def _make_plan(lookups):
    """Decide expert->(core,slot) placement and slot capacities."""
    counts = np.bincount(lookups, minlength=DIMS)
    order = np.argsort(-counts, kind="stable")

    # Expert-sharded: slot s holds experts ranked [8s, 8s+8), one per core.
    caps_es = [int(counts[order[8 * s]]) for s in range(8)]
    t_es = S * sum(caps_es)
    bytes_es = 2 * t_es * IN_DIM * 4 + 8 * IN_DIM * OUT_DIM * 4

    # Data-parallel: every core holds all 64 experts, samples split 8 ways.
    caps_dp = [int(np.ceil(counts[d] / N_CORES)) for d in range(DIMS)]
    t_dp = S * sum(caps_dp)
    bytes_dp = 2 * t_dp * IN_DIM * 4 + DIMS * IN_DIM * OUT_DIM * 4

    by_expert = [np.nonzero(lookups == d)[0] for d in range(DIMS)]
    if bytes_es <= bytes_dp:
        caps = caps_es
        slot_experts = [[int(order[8 * s + c]) for s in range(8)]
                        for c in range(N_CORES)]
        slot_samples = [[by_expert[slot_experts[c][s]] for s in range(8)]
                        for c in range(N_CORES)]
    else:
        caps = caps_dp
        slot_experts = [list(range(DIMS)) for _ in range(N_CORES)]
        slot_samples = [[by_expert[d][c::N_CORES] for d in range(DIMS)]
                        for c in range(N_CORES)]
    return caps, slot_experts, slot_samples


def _build_program(caps):
    """One SPMD Bass program; slot boundaries from `caps` are hardcoded."""
    n_slots = len(caps)
    total = S * sum(caps)  # token columns per core, incl. padding
    segs = []  # (slot, col0, col1)
    off = 0
    for s, cap in enumerate(caps):
        segs.append((s, off, off + S * cap))
        off += S * cap

    nc = bacc.Bacc("TRN2", target_bir_lowering=False, debug=False,
                   num_devices=N_CORES)
    xt = nc.dram_tensor("xt", [IN_DIM, total], F32, kind="ExternalInput")
    wt = nc.dram_tensor("wt", [IN_DIM, n_slots * OUT_DIM], F32,
                        kind="ExternalInput")
    bt = nc.dram_tensor("bt", [OUT_DIM, n_slots], F32, kind="ExternalInput")
    ot = nc.dram_tensor("ot", [OUT_DIM, total], F32, kind="ExternalOutput")

    n_wgroups = (n_slots + 7) // 8
    with tile.TileContext(nc) as tc:
        with (
            tc.tile_pool(name="wpool", bufs=1) as wpool,
            tc.tile_pool(name="xpool", bufs=3) as xpool,
            tc.tile_pool(name="opool", bufs=3) as opool,
            tc.tile_pool(name="pspool", bufs=2, space="PSUM") as pspool,
        ):
            wtiles = []
            for g in range(n_wgroups):
                gw = min(8, n_slots - 8 * g) * OUT_DIM
                wtile = wpool.tile([IN_DIM, gw], F32, tag=f"w{g}")
                nc.sync.dma_start(wtile[:],
                                  wt[:, 8 * g * OUT_DIM:8 * g * OUT_DIM + gw])
                wtiles.append(wtile)
            btile = wpool.tile([OUT_DIM, n_slots], F32, tag="bias")
            nc.sync.dma_start(btile[:], bt[:])

            for c0 in range(0, total, CHUNK):
                c1 = min(c0 + CHUNK, total)
                w = c1 - c0
                xtile = xpool.tile([IN_DIM, w], F32, tag="x")
                nc.sync.dma_start(xtile[:], xt[:, c0:c1])
                ps = pspool.tile([OUT_DIM, w], F32, tag="ps")
                otile = opool.tile([OUT_DIM, w], F32, tag="o")
                # pieces clipped to slot boundaries AND 512-col PSUM banks
                pieces = []
                for s, s0, s1 in segs:
                    lc, rc = max(c0, s0), min(c1, s1)
                    while lc < rc:
                        bend = min(rc, lc - c0 - (lc - c0) % MM_N + MM_N + c0)
                        pieces.append((s, lc - c0, bend - c0))
                        lc = bend
                for s, l, r in pieces:
                    nc.tensor.matmul(
                        ps[:, l:r],
                        wtiles[s // 8][:, (s % 8) * OUT_DIM:(s % 8 + 1) * OUT_DIM],
                        xtile[:, l:r],
                        start=True, stop=True,
                    )
                for s, l, r in pieces:
                    nc.vector.tensor_scalar_add(
                        otile[:, l:r], ps[:, l:r], btile[:, s:s + 1])
                # out-DMA dispatched from the scalar engine's HWDGE ring so
                # it doesn't serialize behind input DMAs on sync
                nc.scalar.dma_start(ot[:, c0:c1], otile[:])
    nc.compile()
    return nc


def _run(values, W, b, lookups, trace=False, tmpdir=None):
    values = np.asarray(values, dtype=np.float32)
    W = np.asarray(W, dtype=np.float32)
    b = np.asarray(b, dtype=np.float32)
    lookups = np.asarray(lookups).astype(np.int64)

    caps, slot_experts, slot_samples = _make_plan(lookups)
    n_slots = len(caps)
    n_samp = sum(caps)  # padded sample rows per core
    total = S * n_samp

    slot_off = np.cumsum([0] + caps[:-1])
    in_maps = []
    placements = []  # per core: (src_idx, dst_rows)
    for c in range(N_CORES):
        Xg = np.zeros((n_samp, S, IN_DIM), np.float32)
        src_all, dst_all = [], []
        for s in range(n_slots):
            idxs = slot_samples[c][s]
            if len(idxs):
                rows = np.arange(slot_off[s], slot_off[s] + len(idxs))
                Xg[rows] = values[idxs]
                src_all.append(idxs)
                dst_all.append(rows)
        src_all = np.concatenate(src_all) if src_all else np.empty(0, np.int64)
        dst_all = np.concatenate(dst_all) if dst_all else np.empty(0, np.int64)
        placements.append((src_all, dst_all))

        experts_c = slot_experts[c]
        wt = np.ascontiguousarray(
            W[experts_c].transpose(1, 0, 2).reshape(IN_DIM, n_slots * OUT_DIM))
        bt = np.ascontiguousarray(b[experts_c].T)
        xt = np.ascontiguousarray(Xg.reshape(total, IN_DIM).T)
        in_maps.append({"xt": xt, "wt": wt, "bt": bt})

    nc = _build_program(caps)
    res = run_bass_kernel_spmd(nc, in_maps, list(range(N_CORES)),
                               trace=trace, tmpdir=tmpdir)

    out = np.empty((B, S, OUT_DIM), np.float32)
    for c in range(N_CORES):
        og = np.ascontiguousarray(res.results[c]["ot"].T).reshape(
            n_samp, S, OUT_DIM)
        src_all, dst_all = placements[c]
        out[src_all] = og[dst_all]
    return out, res


def kernel(values, W, b, lookups):
    out, _ = _run(values, W, b, lookups)
    return out


# revision 10
# speedup vs baseline: 1.4003x; 1.1156x over previous
"""MultiDense / MoE-routing kernel for 8 Trainium2 NeuronCores.

out[b, s, :] = values[b, s, :] @ W[lookups[b]] + bias[lookups[b]]

Strategy: route on host, compute on device.
  * Experts are sorted by sample count and grouped into 8 "slots" of 8
    similar-sized experts; slot s's experts go one-per-core, so each core
    holds only 8 expert weight matrices (0.5 MiB) and ~1/8 of the batch.
  * Per-core activations are gathered into slot-contiguous order and
    pre-transposed to [IN_DIM=128 partitions, tokens] on host so the device
    does dense matmuls only (no gathers, no transposes).
  * Slot capacities (max expert count within a slot group) are baked into
    the compiled Bass program; all 8 cores run the same SPMD program.
Device per chunk of 512 token-columns: DMA in -> per-slot fp32 matmul
(out^T = W_d^T @ X^T accumulated in PSUM) -> bias-add fused with the
PSUM->SBUF copy -> DMA out. Host inverse-permutes the result.
"""

import numpy as np

import concourse.bacc as bacc
import concourse.mybir as mybir
from concourse import bass, tile
from concourse.bass_utils import run_bass_kernel_spmd

B, S, IN_DIM, OUT_DIM, DIMS = 16384, 4, 128, 128, 64
N_CORES = 8
CHUNK = 2048          # token-columns per pipeline chunk (4 PSUM banks)
MM_N = 512            # max fp32 matmul moving free dim = 1 PSUM bank
F32 = mybir.dt.float32


def _make_plan(lookups):
    """Decide expert->(core,slot) placement and slot capacities."""
    counts = np.bincount(lookups, minlength=DIMS)
    order = np.argsort(-counts, kind="stable")

    # Expert-sharded: slot s holds experts ranked [8s, 8s+8), one per core.
    caps_es = [int(counts[order[8 * s]]) for s in range(8)]
    t_es = S * sum(caps_es)
    bytes_es = 2 * t_es * IN_DIM * 4 + 8 * IN_DIM * OUT_DIM * 4

    # Data-parallel: every core holds all 64 experts, samples split 8 ways.
    caps_dp = [int(np.ceil(counts[d] / N_CORES)) for d in range(DIMS)]
    t_dp = S * sum(caps_dp)
    bytes_dp = 2 * t_dp * IN_DIM * 4 + DIMS * IN_DIM * OUT_DIM * 4

    by_expert = [np.nonzero(lookups == d)[0] for d in range(DIMS)]
    if bytes_es <= bytes_dp:
        caps = caps_es
        slot_experts = [[int(order[8 * s + c]) for s in range(8)]
                        for c in range(N_CORES)]
        slot_samples = [[by_expert[slot_experts[c][s]] for s in range(8)]
                        for c in range(N_CORES)]
    else:
        caps = caps_dp
        slot_experts = [list(range(DIMS)) for _ in range(N_CORES)]
        slot_samples = [[by_expert[d][c::N_CORES] for d in range(DIMS)]
                        for c in range(N_CORES)]
    return caps, slot_experts, slot_samples


def _chunk_plan(total):
    """Variable chunk sizes: small first chunk (PE starts sooner), 1024-col
    body, small tail (shorter pipeline drain)."""
    sizes = []
    rem = total
    first = min(512, rem)
    sizes.append(first)
    rem -= first
    while rem > 1536:
        sizes.append(1024)
        rem -= 1024
    if rem > 512:
        sizes.append(rem - 512)
        rem = 512
    if rem:
        sizes.append(rem)
    bounds = []
    off = 0
    for w in sizes:
        bounds.append((off, off + w))
        off += w
    return bounds


def _build_program(caps):
    """One SPMD Bass program; slot boundaries from `caps` are hardcoded.

    Raw bass Block pipeline (no Tile framework — avoids its preamble
    barriers and end-of-kernel semaphore-reset storm):
      sync   : input DMAs (wt, bt, then xt chunks)
      tensor : per-slot fp32 matmuls into PSUM (out^T = W_d^T @ X^T)
      vector : bias-add fused with PSUM->SBUF copy
      scalar : output DMAs (separate HWDGE ring from sync)
    """
    n_slots = len(caps)
    total = S * sum(caps)  # token columns per core, incl. padding
    segs = []  # (slot, col0, col1)
    off = 0
    for s, cap in enumerate(caps):
        if cap:
            segs.append((s, off, off + S * cap))
        off += S * cap

    chunks = _chunk_plan(total)
    C = len(chunks)
    NP = 4        # PSUM ring depth (4 x [128,1024] = 8 banks)
    CW = max(c1 - c0 for c0, c1 in chunks)

    # per-chunk matmul pieces (split at slot boundaries and 512-col banks)
    mm_pieces, vec_pieces = [], []
    for c0, c1 in chunks:
        mm, vec = [], []
        for s, s0, s1 in segs:
            lc, rc = max(c0, s0), min(c1, s1)
            if lc >= rc:
                continue
            vec.append((s, lc - c0, rc - c0))
            while lc < rc:
                bend = min(rc, c0 + ((lc - c0) // MM_N + 1) * MM_N)
                mm.append((s, lc - c0, bend - c0))
                lc = bend
        mm_pieces.append(mm)
        vec_pieces.append(vec)

    nc = bacc.Bacc("TRN2", target_bir_lowering=False, debug=False,
                   num_devices=N_CORES)
    xt = nc.dram_tensor("xt", [IN_DIM, total], F32, kind="ExternalInput")
    wt = nc.dram_tensor("wt", [IN_DIM, n_slots * OUT_DIM], F32,
                        kind="ExternalInput")
    bt = nc.dram_tensor("bt", [OUT_DIM, n_slots], F32, kind="ExternalInput")
    ot = nc.dram_tensor("ot", [OUT_DIM, total], F32, kind="ExternalOutput")

    # Dedicated per-chunk x/out buffers (SBUF is plentiful) — no buffer
    # recycling, so no DMA-completion-ordering hazards on reuse.
    wtbuf = nc.alloc_sbuf_tensor("wtbuf", [IN_DIM, n_slots * OUT_DIM], F32)
    btbuf = nc.alloc_sbuf_tensor("btbuf", [OUT_DIM, n_slots], F32)
    xbufs = [nc.alloc_sbuf_tensor(f"xbuf{j}", [IN_DIM, c1 - c0], F32)
             for j, (c0, c1) in enumerate(chunks)]
    obufs = [nc.alloc_sbuf_tensor(f"obuf{j}", [OUT_DIM, c1 - c0], F32)
             for j, (c0, c1) in enumerate(chunks)]
    psbufs = [nc.alloc_psum_tensor(f"ps{j}", [OUT_DIM, 1024], F32)
              for j in range(NP)]

    # NOTE: HWDGE DMAs from one engine do NOT complete in issue order
    # (small transfers finish before big ones), so a single cumulative
    # completion counter is racy. Every DMA whose completion anyone waits
    # on gets its own semaphore (per-chunk xs[i]; ws for wt+bt together —
    # the >=32 threshold needs both, so it is order-independent).
    with (
        nc.Block() as block,
        nc.semaphore("ws") as ws,       # wt+bt DMA completions (x16)
        nc.semaphore("dout") as dout,   # output-DMA completions (x16)
        nc.semaphore("mmc") as mmc,     # chunks matmul-complete
        nc.semaphore("vecc") as vecc,   # chunks bias-copy-complete
    ):
        xs = [nc.alloc_semaphore(f"xs{i}") for i in range(C)]
        @block.sync
        def _(sync):
            sync.dma_start(wtbuf[:], wt[:]).then_inc(ws, 16)
            sync.dma_start(btbuf[:], bt[:]).then_inc(ws, 16)
            for i, (c0, c1) in enumerate(chunks):
                sync.dma_start(xbufs[i][:], xt[:, c0:c1]).then_inc(xs[i], 16)

        @block.tensor
        def _(tensor):
            tensor.wait_ge(ws, 32)
            for i in range(C):
                tensor.wait_ge(xs[i], 16)
                if i >= NP:
                    tensor.wait_ge(vecc, i - NP + 1)
                for s, l, r in mm_pieces[i]:
                    tensor.matmul(
                        psbufs[i % NP][:, l:r],
                        wtbuf[:, s * OUT_DIM:(s + 1) * OUT_DIM],
                        xbufs[i][:, l:r],
                        start=True, stop=True,
                    )
                # then_inc directly on a MATMUL can fire before the systolic
                # drain lands in PSUM; an explicit drain is the sound signal
                tensor.drain().then_inc(mmc, 1)

        @block.vector
        def _(vector):
            vector.wait_ge(ws, 32)
            for i in range(C):
                vector.wait_ge(mmc, i + 1)
                vec = vec_pieces[i]
                for k, (s, l, r) in enumerate(vec):
                    inst = vector.tensor_scalar_add(
                        obufs[i][:, l:r], psbufs[i % NP][:, l:r],
                        btbuf[:, s:s + 1])
                    if k == len(vec) - 1:
                        inst.then_inc(vecc, 1)

        @block.scalar
        def _(scalar):
            for i, (c0, c1) in enumerate(chunks):
                scalar.wait_ge(vecc, i + 1)
                scalar.dma_start(ot[:, c0:c1], obufs[i][:]).then_inc(dout, 16)
            scalar.wait_ge(dout, 16 * C)

    nc.compile()
    return nc


def _run(values, W, b, lookups, trace=False, tmpdir=None):
    values = np.asarray(values, dtype=np.float32)
    W = np.asarray(W, dtype=np.float32)
    b = np.asarray(b, dtype=np.float32)
    lookups = np.asarray(lookups).astype(np.int64)

    caps, slot_experts, slot_samples = _make_plan(lookups)
    n_slots = len(caps)
    n_samp = sum(caps)  # padded sample rows per core
    total = S * n_samp

    slot_off = np.cumsum([0] + caps[:-1])
    in_maps = []
    placements = []  # per core: (src_idx, dst_rows)
    for c in range(N_CORES):
        Xg = np.zeros((n_samp, S, IN_DIM), np.float32)
        src_all, dst_all = [], []
        for s in range(n_slots):
            idxs = slot_samples[c][s]
            if len(idxs):
                rows = np.arange(slot_off[s], slot_off[s] + len(idxs))
                Xg[rows] = values[idxs]
                src_all.append(idxs)
                dst_all.append(rows)
        src_all = np.concatenate(src_all) if src_all else np.empty(0, np.int64)
        dst_all = np.concatenate(dst_all) if dst_all else np.empty(0, np.int64)
        placements.append((src_all, dst_all))

        experts_c = slot_experts[c]
        wt = np.ascontiguousarray(
            W[experts_c].transpose(1, 0, 2).reshape(IN_DIM, n_slots * OUT_DIM))
        bt = np.ascontiguousarray(b[experts_c].T)
        xt = np.ascontiguousarray(Xg.reshape(total, IN_DIM).T)
        in_maps.append({"xt": xt, "wt": wt, "bt": bt})

    nc = _build_program(caps)
    res = run_bass_kernel_spmd(nc, in_maps, list(range(N_CORES)),
                               trace=trace, tmpdir=tmpdir)

    out = np.empty((B, S, OUT_DIM), np.float32)
    for c in range(N_CORES):
        og = np.ascontiguousarray(res.results[c]["ot"].T).reshape(
            n_samp, S, OUT_DIM)
        src_all, dst_all = placements[c]
        out[src_all] = og[dst_all]
    return out, res


def kernel(values, W, b, lookups):
    out, _ = _run(values, W, b, lookups)
    return out


# revision 11
# speedup vs baseline: 1.5547x; 1.1102x over previous
"""MultiDense / MoE-routing kernel for 8 Trainium2 NeuronCores.

out[b, s, :] = values[b, s, :] @ W[lookups[b]] + bias[lookups[b]]

Strategy: route on host, compute on device.
  * Experts are sorted by sample count and grouped into 8 "slots" of 8
    similar-sized experts; slot s's experts go one-per-core, so each core
    holds only 8 expert weight matrices (0.5 MiB) and ~1/8 of the batch.
  * Per-core activations are gathered into slot-contiguous order and
    pre-transposed to [IN_DIM=128 partitions, tokens] on host so the device
    does dense matmuls only (no gathers, no transposes).
  * Slot capacities (max expert count within a slot group) are baked into
    the compiled Bass program; all 8 cores run the same SPMD program.
Device per chunk of 512 token-columns: DMA in -> per-slot fp32 matmul
(out^T = W_d^T @ X^T accumulated in PSUM) -> bias-add fused with the
PSUM->SBUF copy -> DMA out. Host inverse-permutes the result.
"""

import numpy as np

import concourse.bacc as bacc
import concourse.mybir as mybir
from concourse import bass, tile
from concourse.bass_utils import run_bass_kernel_spmd

B, S, IN_DIM, OUT_DIM, DIMS = 16384, 4, 128, 128, 64
N_CORES = 8
CHUNK = 2048          # token-columns per pipeline chunk (4 PSUM banks)
MM_N = 512            # max fp32 matmul moving free dim = 1 PSUM bank
F32 = mybir.dt.float32


def _make_plan(lookups):
    """Decide expert->(core,slot) placement and slot capacities."""
    counts = np.bincount(lookups, minlength=DIMS)
    order = np.argsort(-counts, kind="stable")

    # Expert-sharded: slot s holds experts ranked [8s, 8s+8), one per core.
    caps_es = [int(counts[order[8 * s]]) for s in range(8)]
    t_es = S * sum(caps_es)
    bytes_es = 2 * t_es * IN_DIM * 4 + 8 * IN_DIM * OUT_DIM * 4

    # Data-parallel: every core holds all 64 experts, samples split 8 ways.
    caps_dp = [int(np.ceil(counts[d] / N_CORES)) for d in range(DIMS)]
    t_dp = S * sum(caps_dp)
    bytes_dp = 2 * t_dp * IN_DIM * 4 + DIMS * IN_DIM * OUT_DIM * 4

    by_expert = [np.nonzero(lookups == d)[0] for d in range(DIMS)]
    if bytes_es <= bytes_dp:
        caps = caps_es
        slot_experts = [[int(order[8 * s + c]) for s in range(8)]
                        for c in range(N_CORES)]
        slot_samples = [[by_expert[slot_experts[c][s]] for s in range(8)]
                        for c in range(N_CORES)]
    else:
        caps = caps_dp
        slot_experts = [list(range(DIMS)) for _ in range(N_CORES)]
        slot_samples = [[by_expert[d][c::N_CORES] for d in range(DIMS)]
                        for c in range(N_CORES)]
    return caps, slot_experts, slot_samples


def _chunk_plan(total):
    """Variable chunk sizes: small first chunk (PE starts sooner), 1024-col
    body, small tail (shorter pipeline drain)."""
    sizes = []
    rem = total
    first = min(512, rem)
    sizes.append(first)
    rem -= first
    while rem > 1536:
        sizes.append(1024)
        rem -= 1024
    if rem > 512:
        sizes.append(rem - 512)
        rem = 512
    if rem:
        sizes.append(rem)
    bounds = []
    off = 0
    for w in sizes:
        bounds.append((off, off + w))
        off += w
    return bounds


def _build_program(caps):
    """One SPMD Bass program; slot boundaries from `caps` are hardcoded.

    Raw bass Block pipeline (no Tile framework — avoids its preamble
    barriers and end-of-kernel semaphore-reset storm):
      sync   : input DMAs (wt, bt, then xt chunks)
      tensor : per-slot fp32 matmuls into PSUM (out^T = W_d^T @ X^T)
      vector : bias-add fused with PSUM->SBUF copy
      scalar : output DMAs (separate HWDGE ring from sync)
    """
    n_slots = len(caps)
    total = S * sum(caps)  # token columns per core, incl. padding
    segs = []  # (slot, col0, col1)
    off = 0
    for s, cap in enumerate(caps):
        if cap:
            segs.append((s, off, off + S * cap))
        off += S * cap

    chunks = _chunk_plan(total)
    C = len(chunks)
    NP = 4        # PSUM ring depth (4 x [128,1024] = 8 banks)
    CW = max(c1 - c0 for c0, c1 in chunks)

    # per-chunk matmul pieces (split at slot boundaries and 512-col banks)
    mm_pieces, vec_pieces = [], []
    for c0, c1 in chunks:
        mm, vec = [], []
        for s, s0, s1 in segs:
            lc, rc = max(c0, s0), min(c1, s1)
            if lc >= rc:
                continue
            vec.append((s, lc - c0, rc - c0))
            while lc < rc:
                bend = min(rc, c0 + ((lc - c0) // MM_N + 1) * MM_N)
                mm.append((s, lc - c0, bend - c0))
                lc = bend
        mm_pieces.append(mm)
        vec_pieces.append(vec)

    nc = bacc.Bacc("TRN2", target_bir_lowering=False, debug=False,
                   num_devices=N_CORES)
    xt = nc.dram_tensor("xt", [IN_DIM, total], F32, kind="ExternalInput")
    wt = nc.dram_tensor("wt", [IN_DIM, n_slots * OUT_DIM], F32,
                        kind="ExternalInput")
    bt = nc.dram_tensor("bt", [OUT_DIM, n_slots], F32, kind="ExternalInput")
    ot = nc.dram_tensor("ot", [OUT_DIM, total], F32, kind="ExternalOutput")

    # Dedicated per-chunk x/out buffers (SBUF is plentiful) — no buffer
    # recycling, so no DMA-completion-ordering hazards on reuse.
    wtbuf = nc.alloc_sbuf_tensor("wtbuf", [IN_DIM, n_slots * OUT_DIM], F32)
    btbuf = nc.alloc_sbuf_tensor("btbuf", [OUT_DIM, n_slots], F32)
    xbufs = [nc.alloc_sbuf_tensor(f"xbuf{j}", [IN_DIM, c1 - c0], F32)
             for j, (c0, c1) in enumerate(chunks)]
    obufs = [nc.alloc_sbuf_tensor(f"obuf{j}", [OUT_DIM, c1 - c0], F32)
             for j, (c0, c1) in enumerate(chunks)]
    psbufs = [nc.alloc_psum_tensor(f"ps{j}", [OUT_DIM, 1024], F32)
              for j in range(NP)]

    # NOTE: HWDGE DMAs from one engine do NOT complete in issue order
    # (small transfers finish before big ones), so a single cumulative
    # completion counter is racy. Every DMA whose completion anyone waits
    # on gets its own semaphore (per-chunk xs[i]; ws for wt+bt together —
    # the >=32 threshold needs both, so it is order-independent).
    with (
        nc.Block() as block,
        nc.semaphore("ws") as ws,       # wt+bt DMA completions (x16)
        nc.semaphore("dout") as dout,   # output-DMA completions (x16)
        nc.semaphore("mmc") as mmc,     # chunks matmul-complete
        nc.semaphore("vecc") as vecc,   # chunks bias-copy-complete
    ):
        xs = [nc.alloc_semaphore(f"xs{i}") for i in range(C)]
        @block.sync
        def _(sync):
            sync.dma_start(wtbuf[:], wt[:]).then_inc(ws, 16)
            sync.dma_start(btbuf[:], bt[:]).then_inc(ws, 16)
            for i, (c0, c1) in enumerate(chunks):
                sync.dma_start(xbufs[i][:], xt[:, c0:c1]).then_inc(xs[i], 16)

        @block.tensor
        def _(tensor):
            tensor.wait_ge(ws, 32)
            for i in range(C):
                tensor.wait_ge(xs[i], 16)
                if i >= NP:
                    tensor.wait_ge(vecc, i - NP + 1)
                mm = mm_pieces[i]
                for k, (s, l, r) in enumerate(mm):
                    inst = tensor.matmul(
                        psbufs[i % NP][:, l:r],
                        wtbuf[:, s * OUT_DIM:(s + 1) * OUT_DIM],
                        xbufs[i][:, l:r],
                        start=True, stop=True,
                    )
                    if k == len(mm) - 1:
                        inst.then_inc(mmc, 1)

        @block.vector
        def _(vector):
            vector.wait_ge(ws, 32)
            for i in range(C):
                vector.wait_ge(mmc, i + 1)
                vec = vec_pieces[i]
                for k, (s, l, r) in enumerate(vec):
                    inst = vector.tensor_scalar_add(
                        obufs[i][:, l:r], psbufs[i % NP][:, l:r],
                        btbuf[:, s:s + 1])
                    if k == len(vec) - 1:
                        inst.then_inc(vecc, 1)

        @block.scalar
        def _(scalar):
            for i, (c0, c1) in enumerate(chunks):
                scalar.wait_ge(vecc, i + 1)
                scalar.dma_start(ot[:, c0:c1], obufs[i][:]).then_inc(dout, 16)
            scalar.wait_ge(dout, 16 * C)

    nc.compile()
    return nc


def _run(values, W, b, lookups, trace=False, tmpdir=None):
    values = np.asarray(values, dtype=np.float32)
    W = np.asarray(W, dtype=np.float32)
    b = np.asarray(b, dtype=np.float32)
    lookups = np.asarray(lookups).astype(np.int64)

    caps, slot_experts, slot_samples = _make_plan(lookups)
    n_slots = len(caps)
    n_samp = sum(caps)  # padded sample rows per core
    total = S * n_samp

    slot_off = np.cumsum([0] + caps[:-1])
    in_maps = []
    placements = []  # per core: (src_idx, dst_rows)
    for c in range(N_CORES):
        Xg = np.zeros((n_samp, S, IN_DIM), np.float32)
        src_all, dst_all = [], []
        for s in range(n_slots):
            idxs = slot_samples[c][s]
            if len(idxs):
                rows = np.arange(slot_off[s], slot_off[s] + len(idxs))
                Xg[rows] = values[idxs]
                src_all.append(idxs)
                dst_all.append(rows)
        src_all = np.concatenate(src_all) if src_all else np.empty(0, np.int64)
        dst_all = np.concatenate(dst_all) if dst_all else np.empty(0, np.int64)
        placements.append((src_all, dst_all))

        experts_c = slot_experts[c]
        wt = np.ascontiguousarray(
            W[experts_c].transpose(1, 0, 2).reshape(IN_DIM, n_slots * OUT_DIM))
        bt = np.ascontiguousarray(b[experts_c].T)
        xt = np.ascontiguousarray(Xg.reshape(total, IN_DIM).T)
        in_maps.append({"xt": xt, "wt": wt, "bt": bt})

    nc = _build_program(caps)
    res = run_bass_kernel_spmd(nc, in_maps, list(range(N_CORES)),
                               trace=trace, tmpdir=tmpdir)

    out = np.empty((B, S, OUT_DIM), np.float32)
    for c in range(N_CORES):
        og = np.ascontiguousarray(res.results[c]["ot"].T).reshape(
            n_samp, S, OUT_DIM)
        src_all, dst_all = placements[c]
        out[src_all] = og[dst_all]
    return out, res


def kernel(values, W, b, lookups):
    out, _ = _run(values, W, b, lookups)
    return out


# revision 13
# speedup vs baseline: 1.5781x; 1.0150x over previous
"""MultiDense / MoE-routing kernel for 8 Trainium2 NeuronCores.

out[b, s, :] = values[b, s, :] @ W[lookups[b]] + bias[lookups[b]]

Strategy: route on host, compute on device.
  * Experts are sorted by sample count and grouped into 8 "slots" of 8
    similar-sized experts; slot s's experts go one-per-core, so each core
    holds only 8 expert weight matrices (0.5 MiB) and ~1/8 of the batch.
  * Per-core activations are gathered into slot-contiguous order and
    pre-transposed to [IN_DIM=128 partitions, tokens] on host so the device
    does dense matmuls only (no gathers, no transposes).
  * Slot capacities (max expert count within a slot group) are baked into
    the compiled Bass program; all 8 cores run the same SPMD program.
Device per chunk of 512 token-columns: DMA in -> per-slot fp32 matmul
(out^T = W_d^T @ X^T accumulated in PSUM) -> bias-add fused with the
PSUM->SBUF copy -> DMA out. Host inverse-permutes the result.
"""

import numpy as np

import concourse.bacc as bacc
import concourse.mybir as mybir
from concourse import bass, tile
from concourse.bass_utils import run_bass_kernel_spmd

B, S, IN_DIM, OUT_DIM, DIMS = 16384, 4, 128, 128, 64
N_CORES = 8
CHUNK = 2048          # token-columns per pipeline chunk (4 PSUM banks)
MM_N = 512            # max fp32 matmul moving free dim = 1 PSUM bank
F32 = mybir.dt.float32


def _make_plan(lookups):
    """Decide expert->(core,slot) placement and slot capacities."""
    counts = np.bincount(lookups, minlength=DIMS)
    order = np.argsort(-counts, kind="stable")

    # Expert-sharded: slot s holds experts ranked [8s, 8s+8), one per core.
    caps_es = [int(counts[order[8 * s]]) for s in range(8)]
    t_es = S * sum(caps_es)
    bytes_es = 2 * t_es * IN_DIM * 4 + 8 * IN_DIM * OUT_DIM * 4

    # Data-parallel: every core holds all 64 experts, samples split 8 ways.
    caps_dp = [int(np.ceil(counts[d] / N_CORES)) for d in range(DIMS)]
    t_dp = S * sum(caps_dp)
    bytes_dp = 2 * t_dp * IN_DIM * 4 + DIMS * IN_DIM * OUT_DIM * 4

    by_expert = [np.nonzero(lookups == d)[0] for d in range(DIMS)]
    if bytes_es <= bytes_dp:
        caps = caps_es
        slot_experts = [[int(order[8 * s + c]) for s in range(8)]
                        for c in range(N_CORES)]
        slot_samples = [[by_expert[slot_experts[c][s]] for s in range(8)]
                        for c in range(N_CORES)]
    else:
        caps = caps_dp
        slot_experts = [list(range(DIMS)) for _ in range(N_CORES)]
        slot_samples = [[by_expert[d][c::N_CORES] for d in range(DIMS)]
                        for c in range(N_CORES)]
    return caps, slot_experts, slot_samples


def _chunk_plan(total):
    """Variable chunk sizes: small first chunk (PE starts sooner), 1024-col
    body, small tail (shorter pipeline drain)."""
    sizes = []
    rem = total
    first = min(512, rem)
    sizes.append(first)
    rem -= first
    while rem > 1536:
        sizes.append(1024)
        rem -= 1024
    if rem > 512:
        sizes.append(rem - 512)
        rem = 512
    if rem:
        sizes.append(rem)
    bounds = []
    off = 0
    for w in sizes:
        bounds.append((off, off + w))
        off += w
    return bounds


def _build_program(caps):
    """One SPMD Bass program; slot boundaries from `caps` are hardcoded.

    Raw bass Block pipeline (no Tile framework — avoids its preamble
    barriers and end-of-kernel semaphore-reset storm):
      sync   : input DMAs (wt, bt, then xt chunks)
      tensor : per-slot fp32 matmuls into PSUM (out^T = W_d^T @ X^T)
      vector : bias-add fused with PSUM->SBUF copy
      scalar : output DMAs (separate HWDGE ring from sync)
    """
    n_slots = len(caps)
    total = S * sum(caps)  # token columns per core, incl. padding
    segs = []  # (slot, col0, col1)
    off = 0
    for s, cap in enumerate(caps):
        if cap:
            segs.append((s, off, off + S * cap))
        off += S * cap

    chunks = _chunk_plan(total)
    C = len(chunks)
    NP = 4        # PSUM ring depth (4 x [128,1024] = 8 banks)
    CW = max(c1 - c0 for c0, c1 in chunks)

    # per-chunk matmul pieces (split at slot boundaries and 512-col banks)
    mm_pieces, vec_pieces = [], []
    for c0, c1 in chunks:
        mm, vec = [], []
        for s, s0, s1 in segs:
            lc, rc = max(c0, s0), min(c1, s1)
            if lc >= rc:
                continue
            vec.append((s, lc - c0, rc - c0))
            while lc < rc:
                bend = min(rc, c0 + ((lc - c0) // MM_N + 1) * MM_N)
                mm.append((s, lc - c0, bend - c0))
                lc = bend
        mm_pieces.append(mm)
        vec_pieces.append(vec)

    nc = bacc.Bacc("TRN2", target_bir_lowering=False, debug=False,
                   num_devices=N_CORES)
    xt = nc.dram_tensor("xt", [IN_DIM, total], F32, kind="ExternalInput")
    wt = nc.dram_tensor("wt", [IN_DIM, n_slots * OUT_DIM], F32,
                        kind="ExternalInput")
    bt = nc.dram_tensor("bt", [OUT_DIM, n_slots], F32, kind="ExternalInput")
    ot = nc.dram_tensor("ot", [OUT_DIM, total], F32, kind="ExternalOutput")

    # Dedicated per-chunk x/out buffers (SBUF is plentiful) — no buffer
    # recycling, so no DMA-completion-ordering hazards on reuse.
    wtbuf = nc.alloc_sbuf_tensor("wtbuf", [IN_DIM, n_slots * OUT_DIM], F32)
    btbuf = nc.alloc_sbuf_tensor("btbuf", [OUT_DIM, n_slots], F32)
    xbufs = [nc.alloc_sbuf_tensor(f"xbuf{j}", [IN_DIM, c1 - c0], F32)
             for j, (c0, c1) in enumerate(chunks)]
    obufs = [nc.alloc_sbuf_tensor(f"obuf{j}", [OUT_DIM, c1 - c0], F32)
             for j, (c0, c1) in enumerate(chunks)]
    psbufs = [nc.alloc_psum_tensor(f"ps{j}", [OUT_DIM, 1024], F32)
              for j in range(NP)]

    # NOTE: HWDGE DMAs from one engine do NOT complete in issue order
    # (small transfers finish before big ones), so a single cumulative
    # completion counter is racy. Every DMA whose completion anyone waits
    # on gets its own semaphore (per-chunk xs[i]; ws for wt+bt together —
    # the >=32 threshold needs both, so it is order-independent).
    with (
        nc.Block() as block,
        nc.semaphore("ws") as ws,       # wt+bt DMA completions (x16)
        nc.semaphore("dout") as dout,   # output-DMA completions (x16)
        nc.semaphore("mmc") as mmc,     # chunks matmul-complete
        nc.semaphore("vecc") as vecc,   # chunks bias-copy-complete
    ):
        xs = [nc.alloc_semaphore(f"xs{i}") for i in range(C)]
        @block.sync
        def _(sync):
            sync.dma_start(wtbuf[:], wt[:]).then_inc(ws, 16)
            sync.dma_start(btbuf[:], bt[:]).then_inc(ws, 16)
            for i, (c0, c1) in enumerate(chunks):
                sync.dma_start(xbufs[i][:], xt[:, c0:c1]).then_inc(xs[i], 16)

        @block.tensor
        def _(tensor):
            tensor.wait_ge(ws, 32)
            for i in range(C):
                tensor.wait_ge(xs[i], 16)
                if i >= NP:
                    tensor.wait_ge(vecc, i - NP + 1)
                mm = mm_pieces[i]
                for k, (s, l, r) in enumerate(mm):
                    inst = tensor.matmul(
                        psbufs[i % NP][:, l:r],
                        wtbuf[:, s * OUT_DIM:(s + 1) * OUT_DIM],
                        xbufs[i][:, l:r],
                        start=True, stop=True,
                    )
                    if k == len(mm) - 1:
                        inst.then_inc(mmc, 1)

        @block.vector
        def _(vector):
            vector.wait_ge(ws, 32)
            for i in range(C):
                vector.wait_ge(mmc, i + 1)
                vec = vec_pieces[i]
                for k, (s, l, r) in enumerate(vec):
                    inst = vector.tensor_scalar_add(
                        obufs[i][:, l:r], psbufs[i % NP][:, l:r],
                        btbuf[:, s:s + 1])
                    if k == len(vec) - 1:
                        inst.then_inc(vecc, 1)

        @block.scalar
        def _(scalar):
            for i, (c0, c1) in enumerate(chunks):
                scalar.wait_ge(vecc, i + 1)
                scalar.dma_start(ot[:, c0:c1], obufs[i][:]).then_inc(dout, 16)
            scalar.wait_ge(dout, 16 * C)

    nc.compile()
    return nc


def _run(values, W, b, lookups, trace=False, tmpdir=None):
    values = np.asarray(values, dtype=np.float32)
    W = np.asarray(W, dtype=np.float32)
    b = np.asarray(b, dtype=np.float32)
    lookups = np.asarray(lookups).astype(np.int64)

    caps, slot_experts, slot_samples = _make_plan(lookups)
    n_slots = len(caps)
    n_samp = sum(caps)  # padded sample rows per core
    total = S * n_samp

    slot_off = np.cumsum([0] + caps[:-1])
    in_maps = []
    placements = []  # per core: (src_idx, dst_rows)
    for c in range(N_CORES):
        Xg = np.zeros((n_samp, S, IN_DIM), np.float32)
        src_all, dst_all = [], []
        for s in range(n_slots):
            idxs = slot_samples[c][s]
            if len(idxs):
                rows = np.arange(slot_off[s], slot_off[s] + len(idxs))
                Xg[rows] = values[idxs]
                src_all.append(idxs)
                dst_all.append(rows)
        src_all = np.concatenate(src_all) if src_all else np.empty(0, np.int64)
        dst_all = np.concatenate(dst_all) if dst_all else np.empty(0, np.int64)
        placements.append((src_all, dst_all))

        experts_c = slot_experts[c]
        wt = np.ascontiguousarray(
            W[experts_c].transpose(1, 0, 2).reshape(IN_DIM, n_slots * OUT_DIM))
        bt = np.ascontiguousarray(b[experts_c].T)
        xt = np.ascontiguousarray(Xg.reshape(total, IN_DIM).T)
        in_maps.append({"xt": xt, "wt": wt, "bt": bt})

    nc = _build_program(caps)
    res = run_bass_kernel_spmd(nc, in_maps, list(range(N_CORES)),
                               trace=trace, tmpdir=tmpdir)

    out = np.empty((B, S, OUT_DIM), np.float32)
    for c in range(N_CORES):
        og = np.ascontiguousarray(res.results[c]["ot"].T).reshape(
            n_samp, S, OUT_DIM)
        src_all, dst_all = placements[c]
        out[src_all] = og[dst_all]
    return out, res


def kernel(values, W, b, lookups):
    out, _ = _run(values, W, b, lookups)
    return out
